# revision 19
# baseline (speedup 1.0000x reference)
"""Llama4-style MoE (T=1024, H=2048, I=4096, E=8, top-1) on 8 trn2 NeuronCores.

Sharding: expert-parallel, all compute in bf16 (fp32 PSUM accumulate).
Core e owns expert e's weights (48 MB bf16) plus a 1/8 I-shard of the
shared expert (6 MB). Host computes top-1 routing (tiny [1024,8] matmul),
scales each routed token by its sigmoid score, packs per-expert tokens
(padded to capacity C), casts everything to bf16, and dispatches.

Device schedule: the routed-expert matmuls are DMA-bound (weights are
streamed once from HBM), the shared-expert matmuls are PE-bound (their
weights + all tokens are SBUF-resident). The program interleaves
shared-expert "units" between routed weight-streaming groups so the DMA
ring and the PE array both stay busy:
  phase A: routed gate/up groups (stream wg/wu)  x  shared gate/up units
  phase B: routed down groups (stream wd)        x  shared down units
DMAs are batched into multi-k-tile transfers ([128, 4, 512] weight
chunks, half-tensor resident loads) because every DMA instruction holds
the issuing sequencer + HWDGE for ~650ns: hundreds of small DMAs would
serialize into more time than the transfers themselves.
Host sums the 8 partial shared outputs and scatters routed rows back.
"""

import numpy as np
import ml_dtypes

T, H, I, E = 1024, 2048, 4096, 8
P = 128
ISH = I // E          # 512  shared-expert I-shard per core
KH = H // P           # 16
MI = I // P           # 32
MH = H // P           # 16
KSH = ISH // P        # 4
TC = 512              # token chunk for shared-expert moving dim

_BASS_CACHE = {}
LAST_RESULT = None    # BassKernelResults of the most recent run (for test harness)
LAST_C = None


def _build_bass(C, wbufs=16):
    import concourse.bass as bass
    import concourse.mybir as mybir
    import concourse.tile as tile

    bf16 = mybir.dt.bfloat16
    f32 = mybir.dt.float32
    SILU = mybir.ActivationFunctionType.Silu
    MULT = mybir.AluOpType.mult

    nc = bass.Bass(trn_type="TRN2", name=f"moe_ep_bf16_c{C}")

    xe_t = nc.dram_tensor("xe_t", [H, C], bf16, kind="ExternalInput")
    wg = nc.dram_tensor("wg", [H, I], bf16, kind="ExternalInput")
    wu = nc.dram_tensor("wu", [H, I], bf16, kind="ExternalInput")
    wd = nc.dram_tensor("wd", [I, H], bf16, kind="ExternalInput")
    x_t = nc.dram_tensor("x_t", [H, T], bf16, kind="ExternalInput")
    wsg = nc.dram_tensor("wsg", [H, ISH], bf16, kind="ExternalInput")
    wsu = nc.dram_tensor("wsu", [H, ISH], bf16, kind="ExternalInput")
    wsd = nc.dram_tensor("wsd", [ISH, H], bf16, kind="ExternalInput")
    ro_t = nc.dram_tensor("ro_t", [H, C], bf16, kind="ExternalOutput")
    sp_t = nc.dram_tensor("sp_t", [H, T], bf16, kind="ExternalOutput")

    with tile.TileContext(nc) as tc:
        from contextlib import ExitStack

        with ExitStack() as ctx:
            const = ctx.enter_context(tc.tile_pool(name="const", bufs=1))
            wpool = ctx.enter_context(tc.tile_pool(name="wpool", bufs=wbufs))
            hbuf = ctx.enter_context(tc.tile_pool(name="hbuf", bufs=3))
            outp = ctx.enter_context(tc.tile_pool(name="outp", bufs=2))
            psum = ctx.enter_context(tc.tile_pool(name="psum", bufs=8, space="PSUM"))

            # --- SBUF-resident tensors ---
            xeT = const.tile([P, KH, C], bf16, name="xeT")        # routed tokens
            x_sb = const.tile([P, KH, T], bf16, name="x_sb")      # all tokens
            wsg_sb = const.tile([P, KH, ISH], bf16, name="wsg_sb")
            wsu_sb = const.tile([P, KH, ISH], bf16, name="wsu_sb")
            wsd_sb = const.tile([P, KSH, H], bf16, name="wsd_sb")
            hT = const.tile([P, MI, C], bf16, name="hT")          # routed hidden
            hs = const.tile([P, KSH, T], bf16, name="hs")         # shared hidden

            xe_view = xe_t.ap().rearrange("(k p) c -> p k c", p=P)
            x_view = x_t.ap().rearrange("(k p) t -> p k t", p=P)
            wg_view = wg.ap().rearrange("(k p) i -> p k i", p=P)
            wu_view = wu.ap().rearrange("(k p) i -> p k i", p=P)
            wd_view = wd.ap().rearrange("(k p) h -> p k h", p=P)
            wsg_view = wsg.ap().rearrange("(k p) i -> p k i", p=P)
            wsu_view = wsu.ap().rearrange("(k p) i -> p k i", p=P)
            wsd_view = wsd.ap().rearrange("(k p) h -> p k h", p=P)
            ro_view = ro_t.ap().rearrange("(m p) c -> p m c", p=P)
            sp_view = sp_t.ap().rearrange("(m p) t -> p m t", p=P)

            # --- resident loads ---
            # DMA priority order = what unlocks PE work soonest: the
            # k-quarters of x(t0)/wsg/wsu feed the split-K shared wavefront
            # (first PE work at ~5us); xe only gates routed g0, which runs
            # much later.
            for q in range(4):
                nc.sync.dma_start(out=x_sb[:, 4 * q:4 * q + 4, 0:TC], in_=x_view[:, 4 * q:4 * q + 4, 0:TC])
                nc.sync.dma_start(out=wsg_sb[:, 4 * q:4 * q + 4, :], in_=wsg_view[:, 4 * q:4 * q + 4, :])
                nc.sync.dma_start(out=wsu_sb[:, 4 * q:4 * q + 4, :], in_=wsu_view[:, 4 * q:4 * q + 4, :])
            nc.sync.dma_start(out=xeT, in_=xe_view)

            def x_t1_residents():
                nc.sync.dma_start(out=x_sb[:, 0:8, TC:T], in_=x_view[:, 0:8, TC:T])
                nc.sync.dma_start(out=x_sb[:, 8:16, TC:T], in_=x_view[:, 8:16, TC:T])

            def wsd_residents():
                nc.sync.dma_start(out=wsd_sb[:, 0:2, :], in_=wsd_view[:, 0:2, :])
                nc.sync.dma_start(out=wsd_sb[:, 2:4, :], in_=wsd_view[:, 2:4, :])

            # ---------- phase A building blocks ----------
            def routed_gu_group(g, post_dma=None, sd_every=0):
                # gate+up for I-columns [g*512, (g+1)*512) of the routed
                # expert; weights arrive as 4-k-tile chunks (1 MB DMAs).
                # sd_every>0 folds a shared-down unit in every sd_every
                # k-steps (late phase-A groups, once hs(t0)/wsd are ready).
                cs = g * 512
                wgcs, wucs = [], []
                for q in range(4):
                    wgc = wpool.tile([P, 4, 512], bf16, tag="wblk", name=f"wgc{g}_{q}")
                    nc.sync.dma_start(out=wgc, in_=wg_view[:, 4 * q:4 * q + 4, cs:cs + 512])
                    wuc = wpool.tile([P, 4, 512], bf16, tag="wblk", name=f"wuc{g}_{q}")
                    nc.sync.dma_start(out=wuc, in_=wu_view[:, 4 * q:4 * q + 4, cs:cs + 512])
                    wgcs.append(wgc)
                    wucs.append(wuc)
                if post_dma is not None:
                    post_dma()
                gps = [psum.tile([P, 2, C], f32, tag="ps", name=f"gps{g}_{h}") for h in range(2)]
                ups = [psum.tile([P, 2, C], f32, tag="ps", name=f"ups{g}_{h}") for h in range(2)]
                for k in range(KH):
                    wgc, wuc = wgcs[k // 4], wucs[k // 4]
                    for mi in range(4):
                        st = dict(start=(k == 0 and mi % 2 == 0), stop=(k == KH - 1))
                        nc.tensor.matmul(gps[mi // 2][:, mi % 2, :], wgc[:, k % 4, mi * P:(mi + 1) * P], xeT[:, k, :], **st)
                    for mi in range(4):
                        st = dict(start=(k == 0 and mi % 2 == 0), stop=(k == KH - 1))
                        nc.tensor.matmul(ups[mi // 2][:, mi % 2, :], wuc[:, k % 4, mi * P:(mi + 1) * P], xeT[:, k, :], **st)
                    if sd_every and k % sd_every == sd_every - 1:
                        shared_down_unit()
                for h in range(2):
                    h_sb = hbuf.tile([P, 2, C], f32, tag="hsb", name=f"hsb{g}_{h}")
                    nc.scalar.activation(out=h_sb, in_=gps[h], func=SILU)
                    nc.vector.tensor_tensor(hT[:, g * 4 + 2 * h: g * 4 + 2 * h + 2, :], h_sb, ups[h], MULT)

            def shared_gu_pair(t, m):
                # shared-expert gate+up for I-shard tile m, token chunk t.
                # wsg/wsu/x are SBUF-resident: pure PE work, no DMA.
                sg = psum.tile([P, TC], f32, tag="ps", name=f"sg{t}_{m}")
                su = psum.tile([P, TC], f32, tag="ps", name=f"su{t}_{m}")
                for k in range(KH):
                    nc.tensor.matmul(sg, wsg_sb[:, k, m * P:(m + 1) * P], x_sb[:, k, t * TC:(t + 1) * TC],
                                     start=(k == 0), stop=(k == KH - 1))
                for k in range(KH):
                    nc.tensor.matmul(su, wsu_sb[:, k, m * P:(m + 1) * P], x_sb[:, k, t * TC:(t + 1) * TC],
                                     start=(k == 0), stop=(k == KH - 1))
                stmp = hbuf.tile([P, TC], f32, tag="stmp", name=f"stmp{t}_{m}")
                nc.scalar.activation(out=stmp, in_=sg, func=SILU)
                nc.vector.tensor_tensor(hs[:, m, t * TC:(t + 1) * TC], stmp, su, MULT)

            sd_units = [(t, m2) for t in range(2) for m2 in range(MH)]  # 32
            _sdi = [0]
            _quad = [None]

            def shared_down_unit():
                if _sdi[0] >= len(sd_units):
                    return
                t, m2 = sd_units[_sdi[0]]
                _sdi[0] += 1
                sps = psum.tile([P, TC], f32, tag="ps", name=f"sps{t}_{m2}")
                for k2 in range(KSH):
                    nc.tensor.matmul(sps, wsd_sb[:, k2, m2 * P:(m2 + 1) * P], hs[:, k2, t * TC:(t + 1) * TC],
                                     start=(k2 == 0), stop=(k2 == KSH - 1))
                if m2 % 4 == 0:
                    _quad[0] = outp.tile([P, 4, TC], bf16, tag="spsb", name=f"spq{t}_{m2}")
                nc.vector.tensor_copy(out=_quad[0][:, m2 % 4, :], in_=sps)
                if m2 % 4 == 3:
                    # out-DMAs ride the Activation queue: a not-yet-ready
                    # output must not block the SP weight stream.
                    nc.scalar.dma_start(out=sp_view[:, m2 - 3:m2 + 1, t * TC:(t + 1) * TC], in_=_quad[0])

            def shared_gu_wavefront_t0():
                # Split-K wavefront over the four t0 shared pairs: emit the
                # matmuls in k-quarter sweeps so each sweep only needs the
                # k-quarter of wsg/wsu/x that has already landed — the PE
                # starts as soon as the first resident DMAs finish. PSUM
                # banks for an accumulation group stay open across the
                # interleave (start on k==0, stop on k==15) — all 8 banks
                # are held until the drains, which is fine because this
                # runs before routed g0 needs any.
                sgs = [psum.tile([P, TC], f32, tag="ps", name=f"wsg_ps{m}") for m in range(4)]
                sus = [psum.tile([P, TC], f32, tag="ps", name=f"wsu_ps{m}") for m in range(4)]
                for ks, ke, tiles, wt in ((0, 4, sgs, wsg_sb), (0, 4, sus, wsu_sb),
                                          (4, 8, sgs, wsg_sb), (4, 8, sus, wsu_sb),
                                          (8, 12, sgs, wsg_sb), (8, 12, sus, wsu_sb),
                                          (12, 16, sgs, wsg_sb), (12, 16, sus, wsu_sb)):
                    for m in range(4):
                        for k in range(ks, ke):
                            nc.tensor.matmul(tiles[m], wt[:, k, m * P:(m + 1) * P],
                                             x_sb[:, k, 0:TC],
                                             start=(k == 0), stop=(k == KH - 1))
                for m in range(4):
                    stmp = hbuf.tile([P, TC], f32, tag="stmp", name=f"wstmp{m}")
                    nc.scalar.activation(out=stmp, in_=sgs[m], func=SILU)
                    nc.vector.tensor_tensor(hs[:, m, 0:TC], stmp, sus[m], MULT)

            # ---------- phase A: interleave ----------
            # PE order: t0 split-K wavefront (ready at ~6us), then routed
            # groups with the t1 pairs spread between them. DMA order:
            # shared residents, xe, g0 chunks, x_t1, g1 chunks, wsd,
            # g2..g7 chunks — the weight stream runs continuously while
            # the PE alternates between DMA-fed routed work and resident
            # shared work.
            shared_gu_wavefront_t0()
            routed_gu_group(0, post_dma=x_t1_residents)
            shared_gu_pair(1, 0)
            routed_gu_group(1, post_dma=wsd_residents)
            shared_gu_pair(1, 1)
            routed_gu_group(2)
            shared_gu_pair(1, 2)
            routed_gu_group(3)
            shared_gu_pair(1, 3)
            for g in range(4, 8):
                routed_gu_group(g)

            # ---------- phase B building blocks ----------
            def routed_down_group(g2):
                # H-columns [g2*512, (g2+1)*512) of the routed down-proj,
                # with the remaining shared-down units folded in every 8
                # k-steps.
                cs = g2 * 512
                wdcs = []
                for q in range(8):
                    wdc = wpool.tile([P, 4, 512], bf16, tag="wblk", name=f"wdc{g2}_{q}")
                    nc.sync.dma_start(out=wdc, in_=wd_view[:, 4 * q:4 * q + 4, cs:cs + 512])
                    wdcs.append(wdc)
                dps = [psum.tile([P, 2, C], f32, tag="ps", name=f"dps{g2}_{h}") for h in range(2)]
                for k2 in range(MI):
                    wdc = wdcs[k2 // 4]
                    for mi in range(4):
                        st = dict(start=(k2 == 0 and mi % 2 == 0), stop=(k2 == MI - 1))
                        nc.tensor.matmul(dps[mi // 2][:, mi % 2, :], wdc[:, k2 % 4, mi * P:(mi + 1) * P], hT[:, k2, :], **st)
                    if k2 % 4 == 0:
                        shared_down_unit()
                rost = outp.tile([P, 4, C], bf16, tag="rosb", name=f"rost{g2}")
                for h in range(2):
                    nc.vector.tensor_copy(out=rost[:, 2 * h:2 * h + 2, :], in_=dps[h])
                nc.scalar.dma_start(out=ro_view[:, g2 * 4:(g2 + 1) * 4, :], in_=rost)

            # ---------- phase B: interleave ----------
            for g2 in range(4):
                routed_down_group(g2)
            while _sdi[0] < len(sd_units):
                shared_down_unit()

    # Split surplus semaphore waits onto InstEventSemaphore carriers
    # (walrus matmul codegen has a 1-wait limit) like bacc does.
    import bass_rust
    bass_rust.generate_event_semaphores(nc)
    return nc


def _get_bass(C):
    if C not in _BASS_CACHE:
        _BASS_CACHE[C] = _build_bass(C)
    return _BASS_CACHE[C]


def kernel(**inputs):
    global LAST_RESULT, LAST_C
    bf = ml_dtypes.bfloat16
    x = np.ascontiguousarray(np.asarray(inputs["x"], dtype=np.float32))
    w_router = np.asarray(inputs["w_router"], dtype=np.float32)
    ws_gate = np.asarray(inputs["ws_gate"], dtype=np.float32)
    ws_up = np.asarray(inputs["ws_up"], dtype=np.float32)
    ws_down = np.asarray(inputs["ws_down"], dtype=np.float32)
    we_gate = np.asarray(inputs["we_gate"], dtype=np.float32)
    we_up = np.asarray(inputs["we_up"], dtype=np.float32)
    we_down = np.asarray(inputs["we_down"], dtype=np.float32)

    # --- top-1 routing on host (tiny) ---
    logits = x @ w_router                      # [T, E]
    top = np.argmax(logits, axis=1)            # [T]
    tv = logits[np.arange(x.shape[0]), top]
    score = (1.0 / (1.0 + np.exp(-tv))).astype(np.float32)
    idxs = [np.nonzero(top == e)[0] for e in range(E)]
    maxn = max(len(i) for i in idxs)
    C = max(128, ((maxn + 15) // 16) * 16)
    LAST_C = C

    nc = _get_bass(C)

    x_t = np.ascontiguousarray(x.T).astype(bf)  # [H, T]
    in_maps = []
    for e in range(E):
        idx = idxs[e]
        xe = np.zeros((C, H), np.float32)
        if len(idx):
            xe[:len(idx)] = x[idx] * score[idx, None]
        in_maps.append({
            "xe_t": np.ascontiguousarray(xe.T).astype(bf),
            "wg": we_gate[e].astype(bf),
            "wu": we_up[e].astype(bf),
            "wd": we_down[e].astype(bf),
            "x_t": x_t,
            "wsg": np.ascontiguousarray(ws_gate[:, e * ISH:(e + 1) * ISH]).astype(bf),
            "wsu": np.ascontiguousarray(ws_up[:, e * ISH:(e + 1) * ISH]).astype(bf),
            "wsd": np.ascontiguousarray(ws_down[e * ISH:(e + 1) * ISH, :]).astype(bf),
        })

    from concourse.bass_utils import run_bass_kernel_spmd
    res = run_bass_kernel_spmd(nc, in_maps, core_ids=list(range(E)))
    LAST_RESULT = res
    outs = res.results

    spT = outs[0]["sp_t"].astype(np.float32)
    for e in range(1, E):
        spT += outs[e]["sp_t"].astype(np.float32)
    out = np.ascontiguousarray(spT.T)          # [T, H]
    for e in range(E):
        idx = idxs[e]
        if len(idx):
            out[idx] += outs[e]["ro_t"][:, :len(idx)].astype(np.float32).T
    return out


# revision 22
# speedup vs baseline: 1.0157x; 1.0157x over previous
"""Llama4-style MoE (T=1024, H=2048, I=4096, E=8, top-1) on 8 trn2 NeuronCores.

Sharding: expert-parallel, all compute in bf16 (fp32 PSUM accumulate).
Core e owns expert e's weights (48 MB bf16) plus a 1/8 I-shard of the
shared expert (6 MB). Host computes top-1 routing (tiny [1024,8] matmul),
scales each routed token by its sigmoid score, packs per-expert tokens
(padded to capacity C), casts everything to bf16, and dispatches.

Device schedule: the routed-expert matmuls are DMA-bound (weights are
streamed once from HBM), the shared-expert matmuls are PE-bound (their
weights + all tokens are SBUF-resident). The program interleaves
shared-expert "units" between routed weight-streaming groups so the DMA
ring and the PE array both stay busy:
  phase A: routed gate/up groups (stream wg/wu)  x  shared gate/up units
  phase B: routed down groups (stream wd)        x  shared down units
DMAs are batched into multi-k-tile transfers ([128, 4, 512] weight
chunks, half-tensor resident loads) because every DMA instruction holds
the issuing sequencer + HWDGE for ~650ns: hundreds of small DMAs would
serialize into more time than the transfers themselves.
Host sums the 8 partial shared outputs and scatters routed rows back.
"""

import numpy as np
import ml_dtypes

T, H, I, E = 1024, 2048, 4096, 8
P = 128
ISH = I // E          # 512  shared-expert I-shard per core
KH = H // P           # 16
MI = I // P           # 32
MH = H // P           # 16
KSH = ISH // P        # 4
TC = 512              # token chunk for shared-expert moving dim

_BASS_CACHE = {}
LAST_RESULT = None    # BassKernelResults of the most recent run (for test harness)
LAST_C = None


def _build_bass(C, wbufs=16):
    import concourse.bass as bass
    import concourse.mybir as mybir
    import concourse.tile as tile

    bf16 = mybir.dt.bfloat16
    f32 = mybir.dt.float32
    SILU = mybir.ActivationFunctionType.Silu
    MULT = mybir.AluOpType.mult

    nc = bass.Bass(trn_type="TRN2", name=f"moe_ep_bf16_c{C}")

    xe_t = nc.dram_tensor("xe_t", [H, C], bf16, kind="ExternalInput")
    wg = nc.dram_tensor("wg", [H, I], bf16, kind="ExternalInput")
    wu = nc.dram_tensor("wu", [H, I], bf16, kind="ExternalInput")
    wd = nc.dram_tensor("wd", [I, H], bf16, kind="ExternalInput")
    x_t = nc.dram_tensor("x_t", [H, T], bf16, kind="ExternalInput")
    wsg = nc.dram_tensor("wsg", [H, ISH], bf16, kind="ExternalInput")
    wsu = nc.dram_tensor("wsu", [H, ISH], bf16, kind="ExternalInput")
    wsd = nc.dram_tensor("wsd", [ISH, H], bf16, kind="ExternalInput")
    ro_t = nc.dram_tensor("ro_t", [H, C], bf16, kind="ExternalOutput")
    sp_t = nc.dram_tensor("sp_t", [H, T], bf16, kind="ExternalOutput")

    with tile.TileContext(nc) as tc:
        from contextlib import ExitStack

        with ExitStack() as ctx:
            const = ctx.enter_context(tc.tile_pool(name="const", bufs=1))
            wpool = ctx.enter_context(tc.tile_pool(name="wpool", bufs=wbufs))
            hbuf = ctx.enter_context(tc.tile_pool(name="hbuf", bufs=3))
            outp = ctx.enter_context(tc.tile_pool(name="outp", bufs=2))
            psum = ctx.enter_context(tc.tile_pool(name="psum", bufs=8, space="PSUM"))

            # --- SBUF-resident tensors ---
            xeT = const.tile([P, KH, C], bf16, name="xeT")        # routed tokens
            x_sb = const.tile([P, KH, T], bf16, name="x_sb")      # all tokens
            wsg_sb = const.tile([P, KH, ISH], bf16, name="wsg_sb")
            wsu_sb = const.tile([P, KH, ISH], bf16, name="wsu_sb")
            wsd_sb = const.tile([P, KSH, H], bf16, name="wsd_sb")
            hT = const.tile([P, MI, C], bf16, name="hT")          # routed hidden
            hs = const.tile([P, KSH, T], bf16, name="hs")         # shared hidden

            xe_view = xe_t.ap().rearrange("(k p) c -> p k c", p=P)
            x_view = x_t.ap().rearrange("(k p) t -> p k t", p=P)
            wg_view = wg.ap().rearrange("(k p) i -> p k i", p=P)
            wu_view = wu.ap().rearrange("(k p) i -> p k i", p=P)
            wd_view = wd.ap().rearrange("(k p) h -> p k h", p=P)
            wsg_view = wsg.ap().rearrange("(k p) i -> p k i", p=P)
            wsu_view = wsu.ap().rearrange("(k p) i -> p k i", p=P)
            wsd_view = wsd.ap().rearrange("(k p) h -> p k h", p=P)
            ro_view = ro_t.ap().rearrange("(m p) c -> p m c", p=P)
            sp_view = sp_t.ap().rearrange("(m p) t -> p m t", p=P)

            # --- resident loads ---
            # DMA priority order = what unlocks PE work soonest: the
            # k-quarters of x(t0)/wsg/wsu feed the split-K shared wavefront
            # (first PE work at ~5us); xe only gates routed g0, which runs
            # much later.
            for q in range(4):
                nc.sync.dma_start(out=x_sb[:, 4 * q:4 * q + 4, 0:TC], in_=x_view[:, 4 * q:4 * q + 4, 0:TC])
                nc.sync.dma_start(out=wsg_sb[:, 4 * q:4 * q + 4, :], in_=wsg_view[:, 4 * q:4 * q + 4, :])
                nc.sync.dma_start(out=wsu_sb[:, 4 * q:4 * q + 4, :], in_=wsu_view[:, 4 * q:4 * q + 4, :])
            nc.sync.dma_start(out=xeT, in_=xe_view)

            def x_t1_residents():
                nc.sync.dma_start(out=x_sb[:, 0:8, TC:T], in_=x_view[:, 0:8, TC:T])
                nc.sync.dma_start(out=x_sb[:, 8:16, TC:T], in_=x_view[:, 8:16, TC:T])

            def wsd_residents():
                nc.sync.dma_start(out=wsd_sb[:, 0:2, :], in_=wsd_view[:, 0:2, :])
                nc.sync.dma_start(out=wsd_sb[:, 2:4, :], in_=wsd_view[:, 2:4, :])

            # ---------- phase A building blocks ----------
            def routed_gu_group(g, post_dma=None, sd_every=0):
                # gate+up for I-columns [g*512, (g+1)*512) of the routed
                # expert; weights arrive as 4-k-tile chunks (1 MB DMAs).
                # sd_every>0 folds a shared-down unit in every sd_every
                # k-steps (late phase-A groups, once hs(t0)/wsd are ready).
                cs = g * 512
                wgcs, wucs = [], []
                for q in range(4):
                    wgc = wpool.tile([P, 4, 512], bf16, tag="wblk", name=f"wgc{g}_{q}")
                    nc.sync.dma_start(out=wgc, in_=wg_view[:, 4 * q:4 * q + 4, cs:cs + 512])
                    wuc = wpool.tile([P, 4, 512], bf16, tag="wblk", name=f"wuc{g}_{q}")
                    nc.sync.dma_start(out=wuc, in_=wu_view[:, 4 * q:4 * q + 4, cs:cs + 512])
                    wgcs.append(wgc)
                    wucs.append(wuc)
                if post_dma is not None:
                    post_dma()
                gps = [psum.tile([P, 2, C], f32, tag="ps", name=f"gps{g}_{h}") for h in range(2)]
                ups = [psum.tile([P, 2, C], f32, tag="ps", name=f"ups{g}_{h}") for h in range(2)]
                for k in range(KH):
                    wgc, wuc = wgcs[k // 4], wucs[k // 4]
                    for mi in range(4):
                        st = dict(start=(k == 0 and mi % 2 == 0), stop=(k == KH - 1))
                        nc.tensor.matmul(gps[mi // 2][:, mi % 2, :], wgc[:, k % 4, mi * P:(mi + 1) * P], xeT[:, k, :], **st)
                    for mi in range(4):
                        st = dict(start=(k == 0 and mi % 2 == 0), stop=(k == KH - 1))
                        nc.tensor.matmul(ups[mi // 2][:, mi % 2, :], wuc[:, k % 4, mi * P:(mi + 1) * P], xeT[:, k, :], **st)
                    if sd_every and k % sd_every == sd_every - 1:
                        shared_down_unit()
                for h in range(2):
                    h_sb = hbuf.tile([P, 2, C], f32, tag="hsb", name=f"hsb{g}_{h}")
                    nc.scalar.activation(out=h_sb, in_=gps[h], func=SILU)
                    nc.vector.tensor_tensor(hT[:, g * 4 + 2 * h: g * 4 + 2 * h + 2, :], h_sb, ups[h], MULT)

            def shared_gu_pair(t, m):
                # shared-expert gate+up for I-shard tile m, token chunk t.
                # wsg/wsu/x are SBUF-resident: pure PE work, no DMA.
                sg = psum.tile([P, TC], f32, tag="ps", name=f"sg{t}_{m}")
                su = psum.tile([P, TC], f32, tag="ps", name=f"su{t}_{m}")
                for k in range(KH):
                    nc.tensor.matmul(sg, wsg_sb[:, k, m * P:(m + 1) * P], x_sb[:, k, t * TC:(t + 1) * TC],
                                     start=(k == 0), stop=(k == KH - 1))
                for k in range(KH):
                    nc.tensor.matmul(su, wsu_sb[:, k, m * P:(m + 1) * P], x_sb[:, k, t * TC:(t + 1) * TC],
                                     start=(k == 0), stop=(k == KH - 1))
                stmp = hbuf.tile([P, TC], f32, tag="stmp", name=f"stmp{t}_{m}")
                nc.scalar.activation(out=stmp, in_=sg, func=SILU)
                nc.vector.tensor_tensor(hs[:, m, t * TC:(t + 1) * TC], stmp, su, MULT)

            sd_units = [(t, m2) for t in range(2) for m2 in range(MH)]  # 32
            _sdi = [0]
            _quad = [None]

            def shared_down_unit():
                if _sdi[0] >= len(sd_units):
                    return
                t, m2 = sd_units[_sdi[0]]
                _sdi[0] += 1
                sps = psum.tile([P, TC], f32, tag="ps", name=f"sps{t}_{m2}")
                for k2 in range(KSH):
                    nc.tensor.matmul(sps, wsd_sb[:, k2, m2 * P:(m2 + 1) * P], hs[:, k2, t * TC:(t + 1) * TC],
                                     start=(k2 == 0), stop=(k2 == KSH - 1))
                if m2 % 4 == 0:
                    _quad[0] = outp.tile([P, 4, TC], bf16, tag="spsb", name=f"spq{t}_{m2}")
                nc.vector.tensor_copy(out=_quad[0][:, m2 % 4, :], in_=sps)
                if m2 % 4 == 3:
                    # out-DMAs ride the Activation queue: a not-yet-ready
                    # output must not block the SP weight stream.
                    nc.scalar.dma_start(out=sp_view[:, m2 - 3:m2 + 1, t * TC:(t + 1) * TC], in_=_quad[0])

            def shared_gu_wavefront_t0():
                # Split-K wavefront over the four t0 shared pairs: emit the
                # matmuls in k-quarter sweeps so each sweep only needs the
                # k-quarter of wsg/wsu/x that has already landed — the PE
                # starts as soon as the first resident DMAs finish. PSUM
                # banks for an accumulation group stay open across the
                # interleave (start on k==0, stop on k==15) — all 8 banks
                # are held until the drains, which is fine because this
                # runs before routed g0 needs any.
                # 128-token sub-chunks: the front is DMA-paced, and every
                # just-in-time semaphore wait resets the PE clock ramp — a
                # post-wait matmul runs at 0.65-1.2GHz, so keep the
                # stall-exposed matmuls small (128 rows, not 512).
                sgs = [psum.tile([P, TC], f32, tag="ps", name=f"wsg_ps{m}") for m in range(4)]
                sus = [psum.tile([P, TC], f32, tag="ps", name=f"wsu_ps{m}") for m in range(4)]
                for ks, ke, tiles, wt in ((0, 4, sgs, wsg_sb), (0, 4, sus, wsu_sb),
                                          (4, 8, sgs, wsg_sb), (4, 8, sus, wsu_sb),
                                          (8, 12, sgs, wsg_sb), (8, 12, sus, wsu_sb),
                                          (12, 16, sgs, wsg_sb), (12, 16, sus, wsu_sb)):
                    for m in range(4):
                        for tc in range(4):
                            for k in range(ks, ke):
                                nc.tensor.matmul(tiles[m][:, tc * P:(tc + 1) * P],
                                                 wt[:, k, m * P:(m + 1) * P],
                                                 x_sb[:, k, tc * P:(tc + 1) * P],
                                                 start=(k == 0 and tc == 0), stop=(k == KH - 1))
                for m in range(4):
                    stmp = hbuf.tile([P, TC], f32, tag="stmp", name=f"wstmp{m}")
                    nc.scalar.activation(out=stmp, in_=sgs[m], func=SILU)
                    nc.vector.tensor_tensor(hs[:, m, 0:TC], stmp, sus[m], MULT)

            # ---------- phase A: interleave ----------
            # PE order: t0 split-K wavefront (ready at ~6us), then routed
            # groups with the t1 pairs spread between them. DMA order:
            # shared residents, xe, g0 chunks, x_t1, g1 chunks, wsd,
            # g2..g7 chunks — the weight stream runs continuously while
            # the PE alternates between DMA-fed routed work and resident
            # shared work.
            shared_gu_wavefront_t0()
            routed_gu_group(0, post_dma=x_t1_residents)
            shared_gu_pair(1, 0)
            routed_gu_group(1, post_dma=wsd_residents)
            shared_gu_pair(1, 1)
            routed_gu_group(2)
            shared_gu_pair(1, 2)
            routed_gu_group(3)
            shared_gu_pair(1, 3)
            for g in range(4, 8):
                routed_gu_group(g)

            # ---------- phase B building blocks ----------
            def routed_down_group(g2):
                # H-columns [g2*512, (g2+1)*512) of the routed down-proj,
                # with a shared-down unit folded in every 4 k-steps.
                cs = g2 * 512
                wdcs = []
                for q in range(8):
                    wdc = wpool.tile([P, 4, 512], bf16, tag="wblk", name=f"wdc{g2}_{q}")
                    nc.sync.dma_start(out=wdc, in_=wd_view[:, 4 * q:4 * q + 4, cs:cs + 512])
                    wdcs.append(wdc)
                dps = [psum.tile([P, 2, C], f32, tag="ps", name=f"dps{g2}_{h}") for h in range(2)]
                for k2 in range(MI):
                    wdc = wdcs[k2 // 4]
                    for mi in range(4):
                        st = dict(start=(k2 == 0 and mi % 2 == 0), stop=(k2 == MI - 1))
                        nc.tensor.matmul(dps[mi // 2][:, mi % 2, :], wdc[:, k2 % 4, mi * P:(mi + 1) * P], hT[:, k2, :], **st)
                    if k2 % 4 == 0:
                        shared_down_unit()
                rost = outp.tile([P, 4, C], bf16, tag="rosb", name=f"rost{g2}")
                for h in range(2):
                    nc.vector.tensor_copy(out=rost[:, 2 * h:2 * h + 2, :], in_=dps[h])
                nc.scalar.dma_start(out=ro_view[:, g2 * 4:(g2 + 1) * 4, :], in_=rost)

            # ---------- phase B: interleave ----------
            for g2 in range(4):
                routed_down_group(g2)
            while _sdi[0] < len(sd_units):
                shared_down_unit()

    # Split surplus semaphore waits onto InstEventSemaphore carriers
    # (walrus matmul codegen has a 1-wait limit) like bacc does.
    import bass_rust
    bass_rust.generate_event_semaphores(nc)
    return nc


def _get_bass(C):
    if C not in _BASS_CACHE:
        _BASS_CACHE[C] = _build_bass(C)
    return _BASS_CACHE[C]


def kernel(**inputs):
    global LAST_RESULT, LAST_C
    bf = ml_dtypes.bfloat16
    x = np.ascontiguousarray(np.asarray(inputs["x"], dtype=np.float32))
    w_router = np.asarray(inputs["w_router"], dtype=np.float32)
    ws_gate = np.asarray(inputs["ws_gate"], dtype=np.float32)
    ws_up = np.asarray(inputs["ws_up"], dtype=np.float32)
    ws_down = np.asarray(inputs["ws_down"], dtype=np.float32)
    we_gate = np.asarray(inputs["we_gate"], dtype=np.float32)
    we_up = np.asarray(inputs["we_up"], dtype=np.float32)
    we_down = np.asarray(inputs["we_down"], dtype=np.float32)

    # --- top-1 routing on host (tiny) ---
    logits = x @ w_router                      # [T, E]
    top = np.argmax(logits, axis=1)            # [T]
    tv = logits[np.arange(x.shape[0]), top]
    score = (1.0 / (1.0 + np.exp(-tv))).astype(np.float32)
    idxs = [np.nonzero(top == e)[0] for e in range(E)]
    maxn = max(len(i) for i in idxs)
    C = max(128, ((maxn + 1) // 2) * 2)
    LAST_C = C

    nc = _get_bass(C)

    x_t = np.ascontiguousarray(x.T).astype(bf)  # [H, T]
    in_maps = []
    for e in range(E):
        idx = idxs[e]
        xe = np.zeros((C, H), np.float32)
        if len(idx):
            xe[:len(idx)] = x[idx] * score[idx, None]
        in_maps.append({
            "xe_t": np.ascontiguousarray(xe.T).astype(bf),
            "wg": we_gate[e].astype(bf),
            "wu": we_up[e].astype(bf),
            "wd": we_down[e].astype(bf),
            "x_t": x_t,
            "wsg": np.ascontiguousarray(ws_gate[:, e * ISH:(e + 1) * ISH]).astype(bf),
            "wsu": np.ascontiguousarray(ws_up[:, e * ISH:(e + 1) * ISH]).astype(bf),
            "wsd": np.ascontiguousarray(ws_down[e * ISH:(e + 1) * ISH, :]).astype(bf),
        })

    from concourse.bass_utils import run_bass_kernel_spmd
    res = run_bass_kernel_spmd(nc, in_maps, core_ids=list(range(E)))
    LAST_RESULT = res
    outs = res.results

    spT = outs[0]["sp_t"].astype(np.float32)
    for e in range(1, E):
        spT += outs[e]["sp_t"].astype(np.float32)
    out = np.ascontiguousarray(spT.T)          # [T, H]
    for e in range(E):
        idx = idxs[e]
        if len(idx):
            out[idx] += outs[e]["ro_t"][:, :len(idx)].astype(np.float32).T
    return out
